# revision 38
# baseline (speedup 1.0000x reference)
import sys

sys.path.insert(0, "/opt/trn_rl_repo")

import numpy as np
import ml_dtypes

from concourse import bass, bacc, tile, mybir
from concourse.bass_utils import run_bass_kernel_spmd

B = 8192
NPG = 50
EPG = 100
N = B * NPG
E = B * EPG
F1, F2, F3 = 78, 156, 312
NCORES = 8
GPC = B // NCORES          # 1024 graphs per core
NPC = GPC * NPG            # 51200 nodes per core
PAIRS = GPC // 2           # 512 graph-pairs per core
GRP = 8                    # pairs per DMA group
NGRP = PAIRS // GRP        # 64 DMA groups

BF16 = mybir.dt.bfloat16
F32 = mybir.dt.float32
FP8 = mybir.dt.float8e4
NP_BF16 = ml_dtypes.bfloat16
NP_FP8 = ml_dtypes.float8_e4m3fn
RELU = mybir.ActivationFunctionType.Relu
IDENT = mybir.ActivationFunctionType.Identity
MAXOP = mybir.AluOpType.max
AXX = mybir.AxisListType.X

_CACHE = {}


def _build_at_pairs(edge_index):
    """Host: normalized GCN adjacency, transposed, pair-block-diagonal,
    with an extra all-ones source row (row 100) used to fold the layer-1
    bias into the aggregation matmul.

    AT[g, s, d] = dinv[src]*dinv[dst] summed over edges, AT[g, i, i] += dinv^2
    so that (A_hat @ H) == (H^T @ AT)^T per graph, matching the reference
    segment_sum formulation exactly.
    """
    src = np.asarray(edge_index[0], dtype=np.int64)
    dst = np.asarray(edge_index[1], dtype=np.int64)
    deg = np.bincount(dst, minlength=N).astype(np.float32) + 1.0
    dinv = 1.0 / np.sqrt(deg)
    norm = (dinv[src] * dinv[dst]).astype(np.float64)
    g = dst // NPG
    sl = src - g * NPG
    dl = dst - g * NPG
    flat = g * (NPG * NPG) + sl * NPG + dl
    at = np.bincount(flat, weights=norm, minlength=B * NPG * NPG)
    at = at.astype(np.float32).reshape(B, NPG, NPG)
    d2 = (dinv * dinv).reshape(B, NPG)
    ii = np.arange(NPG)
    at[:, ii, ii] += d2
    atp = np.zeros((B // 2, 2 * NPG, 2 * NPG), dtype=np.float32)
    atp[:, :NPG, :NPG] = at[0::2]
    atp[:, NPG:, NPG:] = at[1::2]
    # group for DMA batching: [NCORES, NGRP, 100, GRP*100]
    atp = atp.astype(NP_BF16).reshape(NCORES, NGRP, GRP, 100, 100)
    return np.ascontiguousarray(atp.transpose(0, 1, 3, 2, 4)).reshape(
        NCORES, NGRP, 100, GRP * 100
    )


def _tile_p1(x, W1):
    """Host-fold the first GCN linear: p1 = x @ W1, node-major per pair
    group [NGRP, 100, GRP*78]. The b1 bias row (row 100 of the [101,...]
    SBUF tiles) is DMA'd once at startup, not per group."""
    p1 = np.asarray(x, dtype=np.float32) @ np.asarray(W1, dtype=np.float32)
    p1 = p1.astype(NP_BF16).reshape(NCORES, NGRP, GRP, 100, F1)
    return np.ascontiguousarray(p1.transpose(0, 1, 3, 2, 4)).reshape(
        NCORES, NGRP, 100, GRP * F1
    )


def _prep_cell(cell):
    cell = np.asarray(cell, dtype=np.float32)
    nrm = np.sqrt((cell * cell).sum(axis=1, keepdims=True))
    cv = cell / np.maximum(nrm, 1e-12)
    cv = cv.reshape(NCORES, GPC, 954)
    cv = np.ascontiguousarray(cv.transpose(0, 2, 1))  # [NCORES, 954, GPC]
    return cv.reshape(NCORES, 9, 106, GPC).astype(NP_BF16)


def _wchunk(w, kc):
    """[K, M] -> sbuf layout [kchunk_rows, nchunks, M]."""
    K, M = w.shape
    n = K // kc
    return np.ascontiguousarray(
        w.reshape(n, kc, M).transpose(1, 0, 2)
    )


def _bchunk(b, pc):
    """[F] -> [F//pc, pc]: column c holds chunk c of the bias, fp32."""
    return np.ascontiguousarray(b.reshape(pc, -1).T).astype(np.float32)


def _build_program():
    nc = bacc.Bacc("TRN2", target_bir_lowering=False, debug=False)

    def din(name, shape, dt=BF16):
        return nc.dram_tensor(name, list(shape), dt, kind="ExternalInput").ap()

    x1p = din("x1p", (NGRP, 100, GRP * F1))
    x2p = din("x2p", (NGRP, 100, GRP * F1))
    a1p = din("a1p", (NGRP, 100, GRP * 100))
    a2p = din("a2p", (NGRP, 100, GRP * 100))
    cellc = din("cellc", (9, 106, GPC))
    b1row_d = din("b1row", (1, GRP, F1))

    # wc2 rows: 0:78 = Wc2, row 78 = bc2 (pairs with the ones row kept in
    # the a2s stationary), 79:100 = zero K-pad (PE streams faster at K>=96)
    wc2_d = din("wc2", (100, F2))
    # wc3 fp8 DoubleRow stationary: k-chunks on dim1 (Ko=2), m-chunks of
    # 104 padded to 112 so the k-plane byte stride is 16-aligned
    wc3_d = din("wc3", (100, 2, 3, 112), FP8)
    wg1_d = din("wg1", (104, 3, F2))
    wg2_d = din("wg2", (78, 2, 128))
    wr1_d = din("wr1", (106, 9, 512))
    wr2_d = din("wr2", (128, 4, 256))
    wr3_d = din("wr3", (128, 2, 128))
    wf1_d = din("wf1", (128, 3, 256))
    wf2_d = din("wf2", (128, 2, 128))
    wo_d = din("wo", (128, 2))
    ones_d = din("ones", (1, 8, 100))

    bc3_d = din("bc3", (104, 3), F32)
    bg1_d = din("bg1", (78, 2), F32)
    bg2_d = din("bg2", (128, 1), F32)
    br1_d = din("br1", (128, 4), F32)
    br2_d = din("br2", (128, 2), F32)
    br3_d = din("br3", (128, 1), F32)
    bf1_d = din("bf1", (128, 2), F32)
    bf2_d = din("bf2", (128, 1), F32)
    bo_d = din("bo", (2, 1), F32)

    out_d = nc.dram_tensor("outT", [2, GPC], F32, kind="ExternalOutput").ap()

    with tile.TileContext(nc) as tc:
        from contextlib import ExitStack

        with ExitStack() as ctx:
            cpool = ctx.enter_context(tc.tile_pool(name="consts", bufs=1))

            def load(dram, shape, dt=BF16):
                nm = dram.name.split("_")[0]
                t = cpool.tile(list(shape), dt, tag=nm, name=nm)
                nc.sync.dma_start(t[:], dram[:])
                return t

            wc2 = load(wc2_d, (100, F2))
            wc3 = load(wc3_d, (100, 2, 3, 112), FP8)
            wg1 = load(wg1_d, (104, 3, F2))
            wg2 = load(wg2_d, (78, 2, 128))
            wr1 = load(wr1_d, (106, 9, 512))
            wr2 = load(wr2_d, (128, 4, 256))
            wr3 = load(wr3_d, (128, 2, 128))
            wf1 = load(wf1_d, (128, 3, 256))
            wf2 = load(wf2_d, (128, 2, 128))
            wo = load(wo_d, (128, 2))
            bc3 = load(bc3_d, (104, 3), F32)
            bg1 = load(bg1_d, (78, 2), F32)
            bg2 = load(bg2_d, (128, 1), F32)
            br1 = load(br1_d, (128, 4), F32)
            br2 = load(br2_d, (128, 2), F32)
            br3 = load(br3_d, (128, 1), F32)
            bf1 = load(bf1_d, (128, 2), F32)
            bf2 = load(bf2_d, (128, 1), F32)
            bo = load(bo_d, (2, 1), F32)

            # persistent per-branch outputs
            pooled_raw = [
                [
                    cpool.tile([104, GPC], BF16, tag=f"pr{d}{c}", name=f"pr{d}{c}")
                    for c in range(3)
                ]
                for d in range(2)
            ]
            pooled = [
                [
                    cpool.tile([104, GPC], BF16, tag=f"pool{d}{c}", name=f"pool{d}{c}")
                    for c in range(3)
                ]
                for d in range(2)
            ]
            demb = [
                cpool.tile([128, GPC], BF16, tag=f"demb{d}", name=f"demb{d}")
                for d in range(2)
            ]
            c3T = cpool.tile([128, GPC], BF16, tag="c3T", name="c3T")

            # io/mid pools are opened before the cell branch so the first
            # drug groups' DMAs are in flight while the cell MLP computes.
            iop = ctx.enter_context(tc.tile_pool(name="io", bufs=6))
            midp = ctx.enter_context(tc.tile_pool(name="mid", bufs=8))

            # preset pad regions of rotating buffers:
            #  a2s rows 79:100 zero, row 78 ones (bias row feeding wc2's
            #  bc2 row); a3s rows 78:100 zero. memsets start at
            #  partition 64 (DVE base-partition rule); rows 64:78 are
            #  rewritten by the in-loop copies each iteration.
            #  p1t/at aug row 100 (b1 / ones) is DMA'd once per buffer;
            #  the in-loop DMAs only write rows 0:100 (keeping the DMA
            #  outer dim at 100 so descriptors spray across queues).
            for _ in range(8):
                t1 = midp.tile([100, 4, 100], BF16, tag="a2s", name="a2s")
                nc.vector.memset(t1[64:100, :, :], 0.0)
                nc.sync.dma_start(t1[78:79, :, :], ones_d[:, 0:4, :])
                t2 = midp.tile([100, 2, 4, 100], FP8, tag="a3s", name="a3s")
                nc.vector.memset(t2[64:100, :, :, :], 0.0)
            for _ in range(6):
                t1 = iop.tile([101, GRP, F1], BF16, tag="p1t", name="p1t")
                nc.sync.dma_start(t1[100:101, :, :], b1row_d[:])
                t2 = iop.tile([101, GRP, 100], BF16, tag="at", name="at")
                nc.sync.dma_start(t2[100:101, :, :], ones_d[:])

            drug_io = ((x1p, a1p), (x2p, a2p))
            gtiles = {}

            def load_group(gi):
                tiles = []
                for d, (xp, ap) in enumerate(drug_io):
                    p1t = iop.tile([101, GRP, F1], BF16, tag="p1t", name="p1t")
                    nc.sync.dma_start(p1t[0:100, :, :], xp[gi])
                    at = iop.tile([101, GRP, 100], BF16, tag="at", name="at")
                    nc.sync.dma_start(at[0:100, :, :], ap[gi])
                    tiles.append((p1t, at))
                gtiles[gi] = tiles

            # ---------------- cell branch (its DMAs prefetch early and
            # its long accumulation chains warm up the PE) ----
            with tc.tile_pool(name="cellp", bufs=1) as clp, tc.tile_pool(
                name="pscell", bufs=2, space=bass.MemorySpace.PSUM
            ) as cps:
                cell_sb = []
                for k in range(9):
                    t = clp.tile([106, GPC], BF16, tag=f"cell{k}", name=f"cell{k}")
                    nc.sync.dma_start(t[:], cellc[k])
                    cell_sb.append(t)
                # first drug groups' inputs stream in behind cellc while
                # the cell MLP computes
                load_group(0)
                load_group(1)
                c1 = clp.tile([128, 4 * GPC], BF16, tag="c1", name="c1")
                for m in range(4):
                    for n in range(2):
                        ps = cps.tile([128, 512], F32, tag="ps", name="ps")
                        for k in range(9):
                            nc.tensor.matmul(
                                ps[:],
                                wr1[:, k, m * 128 : (m + 1) * 128],
                                cell_sb[k][:, n * 512 : (n + 1) * 512],
                                start=(k == 0),
                                stop=(k == 8),
                            )
                        nc.scalar.activation(
                            c1[:, m * GPC + n * 512 : m * GPC + (n + 1) * 512],
                            ps[:],
                            RELU,
                            bias=br1[:, m : m + 1],
                        )
                c2 = clp.tile([128, 2 * GPC], BF16, tag="c2", name="c2")
                for m in range(2):
                    for n in range(2):
                        ps = cps.tile([128, 512], F32, tag="ps", name="ps")
                        for k in range(4):
                            nc.tensor.matmul(
                                ps[:],
                                wr2[:, k, m * 128 : (m + 1) * 128],
                                c1[:, k * GPC + n * 512 : k * GPC + (n + 1) * 512],
                                start=(k == 0),
                                stop=(k == 3),
                            )
                        nc.scalar.activation(
                            c2[:, m * GPC + n * 512 : m * GPC + (n + 1) * 512],
                            ps[:],
                            RELU,
                            bias=br2[:, m : m + 1],
                        )
                for n in range(2):
                    ps = cps.tile([128, 512], F32, tag="ps", name="ps")
                    for k in range(2):
                        nc.tensor.matmul(
                            ps[:],
                            wr3[:, k, :],
                            c2[:, k * GPC + n * 512 : k * GPC + (n + 1) * 512],
                            start=(k == 0),
                            stop=(k == 1),
                        )
                    nc.scalar.activation(
                        c3T[:, n * 512 : (n + 1) * 512], ps[:], IDENT, bias=br3[:]
                    )

            # ---------------- drug branches ----------------
            # Per quad of 4 pairs, 5 phases (all layers exploit GCN
            # associativity A(HW) == (AH)W to aggregate BEFORE the weight
            # matmul for layers 2/3, which halves PSUM->SBUF copy traffic
            # and lets layer 3's big weight matmul run weight-stationary
            # with a 400-col pair-batched moving operand):
            #   A: r1 = at_aug^T @ p1_aug   (at stationary, nm out, +b1 free)
            #   B: a2 = A h1                (h1 stationary, fm out)
            #   C: p2 = a2s^T @ wc2         (a2s stationary w/ ones row, +b2)
            #   D: a3 = A h2                (h2 stationary, fm out)
            #   E: x3 = wc3^T @ a3s         (wc3 stationary, batched N=400)
            # Pool maxes run on GPSIMD, copies on DVE, relus on ScalarE.
            with tc.tile_pool(
                name="psb", bufs=2, space=bass.MemorySpace.PSUM
            ) as psum:
                pending = []
                for gi in range(NGRP):
                    if gi + 2 < NGRP:
                        load_group(gi + 2)
                    tiles = gtiles.pop(gi)

                    def make_quad(d, q):
                        p1t, at = tiles[d]
                        base = q * 4
                        st = {}

                        def pA_agg1():
                            r1 = psum.tile([100, 4, 100], F32, tag="s1", name="r1")
                            for j in range(4):
                                nc.tensor.matmul(
                                    r1[:, j, 0:F1],
                                    at[:, base + j, :],
                                    p1t[:, base + j, :],
                                    start=True,
                                    stop=True,
                                )
                            h1q = midp.tile([100, 4, F1], BF16, tag="h1q", name="h1q")
                            nc.scalar.activation(
                                h1q[:], r1[:, :, 0:F1], RELU
                            )
                            st["h1q"] = h1q

                        def pB_agg2():
                            h1q = st["h1q"]
                            a2q = psum.tile([100, 4, 100], F32, tag="s1", name="a2q")
                            for j in range(4):
                                nc.tensor.matmul(
                                    a2q[0:F1, j, :],
                                    h1q[:, j, :],
                                    at[0:100, base + j, :],
                                    start=True,
                                    stop=True,
                                )
                            a2s = midp.tile([100, 4, 100], BF16, tag="a2s", name="a2s")
                            nc.scalar.copy(
                                a2s[0:F1, :, :], a2q[0:F1, :, :]
                            )
                            st["a2s"] = a2s

                        def pC_xw2():
                            a2s = st["a2s"]
                            p2 = [
                                psum.tile([100, 2, 2, F1], F32, tag="p2", name="p2")
                                for _ in range(2)
                            ]
                            for j in range(4):
                                nc.tensor.matmul(
                                    p2[j // 2][:, j % 2, :, :],
                                    a2s[:, j, :],
                                    wc2[:],
                                    start=True,
                                    stop=True,
                                )
                            h2q = midp.tile(
                                [100, 4, 2, F1], BF16, tag="h2q", name="h2q"
                            )
                            for t in range(2):
                                nc.scalar.activation(
                                    h2q[:, 2 * t : 2 * t + 2, :, :],
                                    p2[t][:],
                                    RELU,
                                )
                            st["h2q"] = h2q

                        def pD_agg3():
                            h2q = st["h2q"]
                            a3s = midp.tile(
                                [100, 2, 4, 100], FP8, tag="a3s", name="a3s"
                            )
                            for c in range(2):
                                a3q = psum.tile(
                                    [F1, 4, 100], F32, tag="a3", name="a3q"
                                )
                                for j in range(4):
                                    nc.tensor.matmul(
                                        a3q[:, j, :],
                                        h2q[:, j, c, :],
                                        at[0:100, base + j, :],
                                        start=True,
                                        stop=True,
                                    )
                                if c == 0:
                                    nc.scalar.copy(
                                        a3s[0:F1, c, :, :], a3q[:]
                                    )
                                else:
                                    nc.vector.tensor_copy(
                                        a3s[0:F1, c, :, :], a3q[:]
                                    )
                            st["a3s"] = a3s

                        # bound at make_quad time: pE may run during the
                        # NEXT group iteration (deferred), when gi has
                        # advanced
                        goff = 2 * (gi * GRP + base)

                        def pE_xw3():
                            a3s = st["a3s"]
                            for m in range(3):
                                x3 = psum.tile([104, 8, 50], F32, tag="x3", name="x3")
                                nc.tensor.matmul(
                                    x3[:],
                                    wc3[:, :, m, 0:104],
                                    a3s[:],
                                    start=True,
                                    stop=True,
                                    perf_mode=mybir.MatmulPerfMode.DoubleRow,
                                )
                                nc.vector.tensor_reduce(
                                    pooled_raw[d][m][:, goff : goff + 8],
                                    x3[:],
                                    AXX,
                                    MAXOP,
                                )

                        return (pA_agg1, pB_agg2, pC_xw2, pD_agg3, pE_xw3)

                    streams = [make_quad(d, q) for d in range(2) for q in range(2)]
                    ph = list(zip(*streams))
                    # Software pipeline: the previous group's E phases
                    # (x3 matmuls + DVE pool reduces) are interleaved into
                    # this group's A-D slots so the DVE reduce burst never
                    # queues ahead of the copies that gate PE progress,
                    # and the PE always has ready matmul work.
                    EP = pending

                    def runp(i):
                        if i < len(EP):
                            EP[i]()

                    runp(0)
                    for fn in ph[0]:
                        fn()
                    for fn in ph[1]:
                        fn()
                    runp(1)
                    runp(2)
                    for fn in ph[2]:
                        fn()
                    runp(3)
                    for fn in ph[3]:
                        fn()
                    pending = list(ph[4])
                for fn in pending:
                    fn()

            # ---------------- drug FC heads ----------------
            with tc.tile_pool(name="fc", bufs=1) as pool, tc.tile_pool(
                name="psfc", bufs=2, space=bass.MemorySpace.PSUM
            ) as psum:
                # deferred bias+relu of the max-pooled GCN outputs
                # (DVE SBUF->SBUF bf16 runs in the fast 2x/4x mode)
                for d in range(2):
                    for c in range(3):
                        nc.vector.tensor_scalar(
                            pooled[d][c][:],
                            pooled_raw[d][c][:],
                            bc3[:, c : c + 1],
                            0.0,
                            mybir.AluOpType.add,
                            MAXOP,
                        )
                for d in range(2):
                    gfc = pool.tile([78, 2 * GPC], BF16, tag=f"gfc{d}", name=f"gfc{d}")
                    for m in range(2):
                        for n in range(2):
                            ps = psum.tile([78, 512], F32, tag="ps", name="ps")
                            for k in range(3):
                                nc.tensor.matmul(
                                    ps[:],
                                    wg1[:, k, m * 78 : (m + 1) * 78],
                                    pooled[d][k][:, n * 512 : (n + 1) * 512],
                                    start=(k == 0),
                                    stop=(k == 2),
                                )
                            nc.scalar.activation(
                                gfc[:, m * GPC + n * 512 : m * GPC + (n + 1) * 512],
                                ps[:],
                                RELU,
                                bias=bg1[:, m : m + 1],
                            )
                    for n in range(2):
                        ps = psum.tile([128, 512], F32, tag="ps", name="ps")
                        for k in range(2):
                            nc.tensor.matmul(
                                ps[:],
                                wg2[:, k, :],
                                gfc[:, k * GPC + n * 512 : k * GPC + (n + 1) * 512],
                                start=(k == 0),
                                stop=(k == 1),
                            )
                        nc.scalar.activation(
                            demb[d][:, n * 512 : (n + 1) * 512],
                            ps[:],
                            IDENT,
                            bias=bg2[:],
                        )

                # ---------------- head ----------------
                xcs = [demb[0], demb[1], c3T]
                hf1 = pool.tile([128, 2 * GPC], BF16, tag="hf1", name="hf1")
                for m in range(2):
                    for n in range(2):
                        ps = psum.tile([128, 512], F32, tag="ps", name="ps")
                        for k in range(3):
                            nc.tensor.matmul(
                                ps[:],
                                wf1[:, k, m * 128 : (m + 1) * 128],
                                xcs[k][:, n * 512 : (n + 1) * 512],
                                start=(k == 0),
                                stop=(k == 2),
                            )
                        nc.scalar.activation(
                            hf1[:, m * GPC + n * 512 : m * GPC + (n + 1) * 512],
                            ps[:],
                            RELU,
                            bias=bf1[:, m : m + 1],
                        )
                hf2 = pool.tile([128, GPC], BF16, tag="hf2", name="hf2")
                for n in range(2):
                    ps = psum.tile([128, 512], F32, tag="ps", name="ps")
                    for k in range(2):
                        nc.tensor.matmul(
                            ps[:],
                            wf2[:, k, :],
                            hf1[:, k * GPC + n * 512 : k * GPC + (n + 1) * 512],
                            start=(k == 0),
                            stop=(k == 1),
                        )
                    nc.scalar.activation(
                        hf2[:, n * 512 : (n + 1) * 512], ps[:], RELU, bias=bf2[:]
                    )
                osb = pool.tile([2, GPC], F32, tag="osb", name="osb")
                for n in range(2):
                    ps = psum.tile([2, 512], F32, tag="ps", name="ps")
                    nc.tensor.matmul(
                        ps[:],
                        wo[:],
                        hf2[:, n * 512 : (n + 1) * 512],
                        start=True,
                        stop=True,
                    )
                    nc.scalar.activation(
                        osb[:, n * 512 : (n + 1) * 512], ps[:], IDENT, bias=bo[:]
                    )
                nc.sync.dma_start(out_d[:], osb[:])

    nc.compile()
    return nc


def kernel(x1, edge_index1, batch1, x2, edge_index2, batch2, cell,
           Wc1, bc1, Wc2, bc2, Wc3, bc3, Wg1, bg1, Wg2, bg2,
           Wr1, br1, Wr2, br2, Wr3, br3, Wf1, bf1, Wf2, bf2, Wo, bo):
    if "nc" not in _CACHE:
        _CACHE["nc"] = _build_program()
    nc = _CACHE["nc"]

    x1p = _tile_p1(x1, Wc1)
    x2p = _tile_p1(x2, Wc1)
    a1p = _build_at_pairs(edge_index1)
    a2p = _build_at_pairs(edge_index2)
    cellc = _prep_cell(cell)

    bf = lambda a: np.asarray(a, dtype=np.float32).astype(NP_BF16)
    f32 = lambda a: np.asarray(a, dtype=np.float32)

    def padk(a, k=100):
        out = np.zeros((k,) + a.shape[1:], dtype=a.dtype)
        out[: a.shape[0]] = a
        return out

    # wc2 with the bias row at k=78
    wc2h = np.zeros((100, F2), dtype=NP_BF16)
    wc2h[0:F1] = bf(Wc2)
    wc2h[F1] = bf(bc2)

    # wc3 fp8 DoubleRow layout [100, 2, 3, 112]
    w3 = np.asarray(Wc3, np.float32)  # [156, 312]
    wc3h = np.zeros((100, 2, 3, 112), dtype=NP_FP8)
    for c in range(2):
        for m in range(3):
            wc3h[0:F1, c, m, 0:104] = w3[
                c * F1 : (c + 1) * F1, 104 * m : 104 * m + 104
            ].astype(NP_FP8)

    shared = dict(
        wc2=wc2h,
        wc3=wc3h,
        wg1=bf(_wchunk(np.asarray(Wg1, np.float32), 104)),
        wg2=bf(_wchunk(np.asarray(Wg2, np.float32), 78)),
        wr1=bf(_wchunk(np.asarray(Wr1, np.float32), 106)),
        wr2=bf(_wchunk(np.asarray(Wr2, np.float32), 128)),
        wr3=bf(_wchunk(np.asarray(Wr3, np.float32), 128)),
        wf1=bf(_wchunk(np.asarray(Wf1, np.float32), 128)),
        wf2=bf(_wchunk(np.asarray(Wf2, np.float32), 128)),
        wo=bf(Wo),
        ones=np.ones((1, 8, 100), dtype=NP_BF16),
        b1row=np.broadcast_to(
            np.asarray(bc1, np.float32).astype(NP_BF16)[None, None, :],
            (1, GRP, F1),
        ).copy(),
        bc3=_bchunk(f32(bc3), 3),
        bg1=_bchunk(f32(bg1), 2),
        bg2=f32(bg2).reshape(128, 1),
        br1=_bchunk(f32(br1), 4),
        br2=_bchunk(f32(br2), 2),
        br3=f32(br3).reshape(128, 1),
        bf1=_bchunk(f32(bf1), 2),
        bf2=f32(bf2).reshape(128, 1),
        bo=f32(bo).reshape(2, 1),
    )

    in_maps = []
    for c in range(NCORES):
        m = dict(shared)
        m["x1p"] = x1p[c]
        m["x2p"] = x2p[c]
        m["a1p"] = a1p[c]
        m["a2p"] = a2p[c]
        m["cellc"] = cellc[c]
        in_maps.append(m)

    res = run_bass_kernel_spmd(nc, in_maps, list(range(NCORES)))
    _CACHE["last_result"] = res
    out = np.concatenate(
        [np.asarray(res.results[c]["outT"], np.float32).T for c in range(NCORES)],
        axis=0,
    )
    return out


# revision 54
# speedup vs baseline: 1.2800x; 1.2800x over previous
import sys

sys.path.insert(0, "/opt/trn_rl_repo")

import numpy as np
import ml_dtypes

from concourse import bass, bacc, tile, mybir
from concourse.bass_utils import run_bass_kernel_spmd

B = 8192
NPG = 50
EPG = 100
N = B * NPG
E = B * EPG
F1, F2, F3 = 78, 156, 312
NCORES = 8
GPC = B // NCORES          # 1024 graphs per core
NPC = GPC * NPG            # 51200 nodes per core
PAIRS = GPC // 2           # 512 graph-pairs per core
GRP = 8                    # pairs per DMA group
NGRP = PAIRS // GRP        # 64 DMA groups

BF16 = mybir.dt.bfloat16
F32 = mybir.dt.float32
FP8 = mybir.dt.float8e4
NP_BF16 = ml_dtypes.bfloat16
NP_FP8 = ml_dtypes.float8_e4m3fn
RELU = mybir.ActivationFunctionType.Relu
IDENT = mybir.ActivationFunctionType.Identity
MAXOP = mybir.AluOpType.max
AXX = mybir.AxisListType.X

_CACHE = {}


def _build_at_pairs(edge_index):
    """Host: normalized GCN adjacency, transposed, pair-block-diagonal,
    with an extra all-ones source row (row 100) used to fold the layer-1
    bias into the aggregation matmul.

    AT[g, s, d] = dinv[src]*dinv[dst] summed over edges, AT[g, i, i] += dinv^2
    so that (A_hat @ H) == (H^T @ AT)^T per graph, matching the reference
    segment_sum formulation exactly.
    """
    src = np.asarray(edge_index[0], dtype=np.int64)
    dst = np.asarray(edge_index[1], dtype=np.int64)
    deg = np.bincount(dst, minlength=N).astype(np.float32) + 1.0
    dinv = 1.0 / np.sqrt(deg)
    norm = (dinv[src] * dinv[dst]).astype(np.float64)
    g = dst // NPG
    sl = src - g * NPG
    dl = dst - g * NPG
    flat = g * (NPG * NPG) + sl * NPG + dl
    at = np.bincount(flat, weights=norm, minlength=B * NPG * NPG)
    at = at.astype(np.float32).reshape(B, NPG, NPG)
    d2 = (dinv * dinv).reshape(B, NPG)
    ii = np.arange(NPG)
    at[:, ii, ii] += d2
    atp = np.zeros((B // 2, 2 * NPG, 2 * NPG), dtype=np.float32)
    atp[:, :NPG, :NPG] = at[0::2]
    atp[:, NPG:, NPG:] = at[1::2]
    # group for DMA batching: [NCORES, NGRP, 100, GRP*100]
    atp = atp.astype(NP_BF16).reshape(NCORES, NGRP, GRP, 100, 100)
    return np.ascontiguousarray(atp.transpose(0, 1, 3, 2, 4)).reshape(
        NCORES, NGRP, 100, GRP * 100
    )


def _pack_group(x, W1, edge_index):
    """Host: one combined per-group tensor [NC, NGRP, 100, GRP, 178]:
    cols 0:78 of pair j = p1 = x @ W1 (node-major), cols 78:178 = the
    normalized pair-block adjacency (transposed). One DMA per drug per
    group with large contiguous rows. The aug row 100 (b1 | ones) of the
    [101,...] SBUF tiles is DMA'd once at startup, not per group."""
    p1 = np.asarray(x, dtype=np.float32) @ np.asarray(W1, dtype=np.float32)
    p1 = p1.astype(NP_BF16).reshape(NCORES, NGRP, GRP, 100, F1)
    p1 = p1.transpose(0, 1, 3, 2, 4)  # [NC, NGRP, 100, GRP, 78]
    at = _build_at_pairs(edge_index)  # [NC, NGRP, 100, GRP*100]
    at = at.reshape(NCORES, NGRP, 100, GRP, 100)
    out = np.empty((NCORES, NGRP, 100, GRP, 178), dtype=NP_BF16)
    out[..., 0:F1] = p1
    out[..., F1:178] = at
    return out


def _prep_cell(cell):
    cell = np.asarray(cell, dtype=np.float32)
    nrm = np.sqrt((cell * cell).sum(axis=1, keepdims=True))
    cv = cell / np.maximum(nrm, 1e-12)
    cv = cv.reshape(NCORES, GPC, 954)
    cv = np.ascontiguousarray(cv.transpose(0, 2, 1))  # [NCORES, 954, GPC]
    cv = cv.reshape(NCORES, 9, 106, GPC).astype(NP_BF16)
    # [NC, 106, 9, GPC] so a single DMA fills the one [106, 9, GPC] tile
    return np.ascontiguousarray(cv.transpose(0, 2, 1, 3))


def _wchunk(w, kc):
    """[K, M] -> sbuf layout [kchunk_rows, nchunks, M]."""
    K, M = w.shape
    n = K // kc
    return np.ascontiguousarray(
        w.reshape(n, kc, M).transpose(1, 0, 2)
    )


def _bchunk(b, pc):
    """[F] -> [F//pc, pc]: column c holds chunk c of the bias, fp32."""
    return np.ascontiguousarray(b.reshape(pc, -1).T).astype(np.float32)


def _build_program():
    nc = bacc.Bacc("TRN2", target_bir_lowering=False, debug=False)

    def din(name, shape, dt=BF16):
        return nc.dram_tensor(name, list(shape), dt, kind="ExternalInput").ap()

    g1p = din("g1p", (NGRP, 100, GRP, 178))
    g2p = din("g2p", (NGRP, 100, GRP, 178))
    cellc = din("cellc", (106, 9, GPC))
    # aug row: per pair j, cols 0:78 = b1, cols 78:178 = ones
    augrow_d = din("augrow", (1, GRP, 178))

    # wc2 rows: 0:78 = Wc2, row 78 = bc2 (pairs with the ones row kept in
    # the a2s stationary), 79:100 = zero K-pad (PE streams faster at K>=96)
    wc2_d = din("wc2", (100, F2))
    # wc3 k-chunks [100, 2, 312]: rows 0:78 real, 78:100 zero
    wc3_d = din("wc3", (100, 2, F3))
    wg1_d = din("wg1", (104, 3, F2))
    wg2_d = din("wg2", (78, 2, 128))
    wr1_d = din("wr1", (106, 9, 512))
    wr2_d = din("wr2", (128, 4, 256))
    wr3_d = din("wr3", (128, 2, 128))
    wf1_d = din("wf1", (128, 3, 256))
    wf2_d = din("wf2", (128, 2, 128))
    wo_d = din("wo", (128, 2))
    ones_d = din("ones", (1, 8, 100))

    bc3_d = din("bc3", (104, 3), F32)
    bg1_d = din("bg1", (78, 2), F32)
    bg2_d = din("bg2", (128, 1), F32)
    br1_d = din("br1", (128, 4), F32)
    br2_d = din("br2", (128, 2), F32)
    br3_d = din("br3", (128, 1), F32)
    bf1_d = din("bf1", (128, 2), F32)
    bf2_d = din("bf2", (128, 1), F32)
    bo_d = din("bo", (2, 1), F32)

    out_d = nc.dram_tensor("outT", [2, GPC], F32, kind="ExternalOutput").ap()

    with tile.TileContext(nc) as tc:
        from contextlib import ExitStack

        with ExitStack() as ctx:
            cpool = ctx.enter_context(tc.tile_pool(name="consts", bufs=1))

            def load(dram, shape, dt=BF16):
                nm = dram.name.split("_")[0]
                t = cpool.tile(list(shape), dt, tag=nm, name=nm)
                nc.sync.dma_start(t[:], dram[:])
                return t

            # DMA issue order is serial on the sync sequencer (~0.6us per
            # dma_start): load the cell branch's inputs first so its PE
            # warm-up starts ASAP, then the first drug groups, then the
            # rest of the constants.
            cellt = cpool.tile([106, 9, GPC], BF16, tag="cellt", name="cellt")
            nc.sync.dma_start(cellt[:], cellc[:])
            wr1 = load(wr1_d, (106, 9, 512))
            wr2 = load(wr2_d, (128, 4, 256))
            wr3 = load(wr3_d, (128, 2, 128))
            br1 = load(br1_d, (128, 4), F32)
            br2 = load(br2_d, (128, 2), F32)
            br3 = load(br3_d, (128, 1), F32)

            # persistent per-branch outputs
            pooled_raw = [
                [
                    cpool.tile([104, GPC], BF16, tag=f"pr{d}{c}", name=f"pr{d}{c}")
                    for c in range(3)
                ]
                for d in range(2)
            ]
            pooled = [
                [
                    cpool.tile([104, GPC], BF16, tag=f"pool{d}{c}", name=f"pool{d}{c}")
                    for c in range(3)
                ]
                for d in range(2)
            ]
            demb = [
                cpool.tile([128, GPC], BF16, tag=f"demb{d}", name=f"demb{d}")
                for d in range(2)
            ]
            c3T = cpool.tile([128, GPC], BF16, tag="c3T", name="c3T")

            # io/mid pools are opened before the cell branch so the first
            # drug groups' DMAs are in flight while the cell MLP computes.
            iop = ctx.enter_context(tc.tile_pool(name="io", bufs=6))
            midp = ctx.enter_context(tc.tile_pool(name="mid", bufs=8))

            drug_io = (g1p, g2p)
            gtiles = {}

            def load_group(gi):
                tiles = []
                for gp in drug_io:
                    ga = iop.tile([101, GRP, 178], BF16, tag="ga", name="ga")
                    nc.sync.dma_start(ga[0:100, :, :], gp[gi])
                    tiles.append(ga)
                gtiles[gi] = tiles

            # ga aug row 100 (b1 | ones per pair) is DMA'd once per
            # rotating buffer BEFORE any group load claims the buffers;
            # the in-loop DMAs only write rows 0:100.
            for _ in range(6):
                t1 = iop.tile([101, GRP, 178], BF16, tag="ga", name="ga")
                nc.sync.dma_start(t1[100:101, :, :], augrow_d[:])

            load_group(0)
            load_group(1)

            # remaining constants (needed only once the drug loop starts)
            wc2 = load(wc2_d, (100, F2))
            wc3 = load(wc3_d, (100, 2, F3))
            wg1 = load(wg1_d, (104, 3, F2))
            wg2 = load(wg2_d, (78, 2, 128))
            wf1 = load(wf1_d, (128, 3, 256))
            wf2 = load(wf2_d, (128, 2, 128))
            wo = load(wo_d, (128, 2))
            bc3 = load(bc3_d, (104, 3), F32)
            bg1 = load(bg1_d, (78, 2), F32)
            bg2 = load(bg2_d, (128, 1), F32)
            bf1 = load(bf1_d, (128, 2), F32)
            bf2 = load(bf2_d, (128, 1), F32)
            bo = load(bo_d, (2, 1), F32)

            # preset pad regions of rotating buffers:
            #  a2s rows 79:100 zero, row 78 ones (bias row feeding wc2's
            #  bc2 row); a3s rows 78:100 zero. memsets start at
            #  partition 64 (DVE base-partition rule); rows 64:78 are
            #  rewritten by the in-loop copies each iteration.
            #  ga aug row 100 (b1 | ones per pair) is DMA'd once per
            #  buffer; the in-loop DMAs only write rows 0:100 (keeping the
            #  DMA outer dim at 100 so descriptors spray across queues).
            for _ in range(8):
                t1 = midp.tile([100, 4, 100], BF16, tag="a2s", name="a2s")
                nc.vector.memset(t1[64:100, :, :], 0.0)
                nc.sync.dma_start(t1[78:79, :, :], ones_d[:, 0:4, :])
                t2 = midp.tile([100, 2, 4, 100], BF16, tag="a3s", name="a3s")
                nc.vector.memset(t2[64:100, :, :, :], 0.0)

            # ---------------- cell branch (its DMAs prefetch first and
            # its long accumulation chains warm up the PE) ----
            with tc.tile_pool(name="cellp", bufs=1) as clp, tc.tile_pool(
                name="pscell", bufs=2, space=bass.MemorySpace.PSUM
            ) as cps:
                c1 = clp.tile([128, 4 * GPC], BF16, tag="c1", name="c1")
                for m in range(4):
                    for n in range(2):
                        ps = cps.tile([128, 512], F32, tag="ps", name="ps")
                        for k in range(9):
                            nc.tensor.matmul(
                                ps[:],
                                wr1[:, k, m * 128 : (m + 1) * 128],
                                cellt[:, k, n * 512 : (n + 1) * 512],
                                start=(k == 0),
                                stop=(k == 8),
                            )
                        nc.scalar.activation(
                            c1[:, m * GPC + n * 512 : m * GPC + (n + 1) * 512],
                            ps[:],
                            RELU,
                            bias=br1[:, m : m + 1],
                        )
                c2 = clp.tile([128, 2 * GPC], BF16, tag="c2", name="c2")
                for m in range(2):
                    for n in range(2):
                        ps = cps.tile([128, 512], F32, tag="ps", name="ps")
                        for k in range(4):
                            nc.tensor.matmul(
                                ps[:],
                                wr2[:, k, m * 128 : (m + 1) * 128],
                                c1[:, k * GPC + n * 512 : k * GPC + (n + 1) * 512],
                                start=(k == 0),
                                stop=(k == 3),
                            )
                        nc.scalar.activation(
                            c2[:, m * GPC + n * 512 : m * GPC + (n + 1) * 512],
                            ps[:],
                            RELU,
                            bias=br2[:, m : m + 1],
                        )
                for n in range(2):
                    ps = cps.tile([128, 512], F32, tag="ps", name="ps")
                    for k in range(2):
                        nc.tensor.matmul(
                            ps[:],
                            wr3[:, k, :],
                            c2[:, k * GPC + n * 512 : k * GPC + (n + 1) * 512],
                            start=(k == 0),
                            stop=(k == 1),
                        )
                    nc.scalar.activation(
                        c3T[:, n * 512 : (n + 1) * 512], ps[:], IDENT, bias=br3[:]
                    )

            # ---------------- drug branches ----------------
            # Per quad of 4 pairs, 5 phases (all layers exploit GCN
            # associativity A(HW) == (AH)W to aggregate BEFORE the weight
            # matmul for layers 2/3, which halves PSUM->SBUF copy traffic
            # and lets layer 3's big weight matmul run weight-stationary
            # with a 400-col pair-batched moving operand):
            #   A: r1 = at_aug^T @ p1_aug   (at stationary, nm out, +b1 free)
            #   B: a2 = A h1                (h1 stationary, fm out)
            #   C: p2 = a2s^T @ wc2         (a2s stationary w/ ones row, +b2)
            #   D: a3 = A h2                (h2 stationary, fm out)
            #   E: x3 = wc3^T @ a3s         (wc3 stationary, batched N=400)
            # Pool maxes run on GPSIMD, copies on DVE, relus on ScalarE.
            with tc.tile_pool(
                name="psb", bufs=2, space=bass.MemorySpace.PSUM
            ) as psum:
                pending = []
                for gi in range(NGRP):
                    if gi + 2 < NGRP:
                        load_group(gi + 2)
                    tiles = gtiles.pop(gi)

                    def make_quad(d, q):
                        ga = tiles[d]
                        base = q * 4
                        st = {}

                        def pA_agg1():
                            r1 = psum.tile([100, 4, 100], F32, tag="s1", name="r1")
                            for j in range(4):
                                nc.tensor.matmul(
                                    r1[:, j, 0:F1],
                                    ga[:, base + j, F1:178],
                                    ga[:, base + j, 0:F1],
                                    start=True,
                                    stop=True,
                                )
                            h1q = midp.tile([100, 4, F1], BF16, tag="h1q", name="h1q")
                            nc.scalar.activation(
                                h1q[:], r1[:, :, 0:F1], RELU
                            )
                            st["h1q"] = h1q

                        def pB_agg2():
                            h1q = st["h1q"]
                            a2q = psum.tile([100, 4, 100], F32, tag="s1", name="a2q")
                            for j in range(4):
                                nc.tensor.matmul(
                                    a2q[0:F1, j, :],
                                    h1q[:, j, :],
                                    ga[0:100, base + j, F1:178],
                                    start=True,
                                    stop=True,
                                )
                            a2s = midp.tile([100, 4, 100], BF16, tag="a2s", name="a2s")
                            nc.scalar.copy(
                                a2s[0:F1, :, :], a2q[0:F1, :, :]
                            )
                            st["a2s"] = a2s

                        def pC_xw2():
                            a2s = st["a2s"]
                            p2 = [
                                psum.tile([100, 2, 2, F1], F32, tag="p2", name="p2")
                                for _ in range(2)
                            ]
                            for j in range(4):
                                nc.tensor.matmul(
                                    p2[j // 2][:, j % 2, :, :],
                                    a2s[:, j, :],
                                    wc2[:],
                                    start=True,
                                    stop=True,
                                )
                            h2q = midp.tile(
                                [100, 4, 2, F1], BF16, tag="h2q", name="h2q"
                            )
                            for t in range(2):
                                nc.scalar.activation(
                                    h2q[:, 2 * t : 2 * t + 2, :, :],
                                    p2[t][:],
                                    RELU,
                                )
                            st["h2q"] = h2q

                        def pD_agg3():
                            h2q = st["h2q"]
                            a3s = midp.tile(
                                [100, 2, 4, 100], BF16, tag="a3s", name="a3s"
                            )
                            for c in range(2):
                                a3q = psum.tile(
                                    [F1, 4, 100], F32, tag="a3", name="a3q"
                                )
                                for j in range(4):
                                    nc.tensor.matmul(
                                        a3q[:, j, :],
                                        h2q[:, j, c, :],
                                        ga[0:100, base + j, F1:178],
                                        start=True,
                                        stop=True,
                                    )
                                if c == 0:
                                    nc.scalar.copy(
                                        a3s[0:F1, c, :, :], a3q[:]
                                    )
                                else:
                                    nc.vector.tensor_copy(
                                        a3s[0:F1, c, :, :], a3q[:]
                                    )
                            st["a3s"] = a3s

                        # bound at make_quad time: pE may run during the
                        # NEXT group iteration (deferred), when gi has
                        # advanced
                        goff = 2 * (gi * GRP + base)

                        def pE_xw3():
                            a3s = st["a3s"]
                            for m in range(3):
                                x3 = psum.tile([104, 8, 50], F32, tag="x3", name="x3")
                                for c in range(2):
                                    nc.tensor.matmul(
                                        x3[:],
                                        wc3[:, c, 104 * m : 104 * m + 104],
                                        a3s[:, c, :, :],
                                        start=(c == 0),
                                        stop=(c == 1),
                                    )
                                nc.vector.tensor_reduce(
                                    pooled_raw[d][m][:, goff : goff + 8],
                                    x3[:],
                                    AXX,
                                    MAXOP,
                                )

                        return (pA_agg1, pB_agg2, pC_xw2, pD_agg3, pE_xw3)

                    streams = [make_quad(d, q) for d in range(2) for q in range(2)]
                    ph = list(zip(*streams))
                    # Software pipeline: the previous group's E phases
                    # (x3 matmuls + DVE pool reduces) are interleaved into
                    # this group's A-D slots so the DVE reduce burst never
                    # queues ahead of the copies that gate PE progress,
                    # and the PE always has ready matmul work.
                    EP = pending

                    def runp(i):
                        if i < len(EP):
                            EP[i]()

                    runp(0)
                    for fn in ph[0]:
                        fn()
                    for fn in ph[1]:
                        fn()
                    runp(1)
                    runp(2)
                    for fn in ph[2]:
                        fn()
                    runp(3)
                    for fn in ph[3]:
                        fn()
                    pending = list(ph[4])
                for fn in pending:
                    fn()

            # ---------------- drug FC heads ----------------
            with tc.tile_pool(name="fc", bufs=1) as pool, tc.tile_pool(
                name="psfc", bufs=2, space=bass.MemorySpace.PSUM
            ) as psum:
                # deferred bias+relu of the max-pooled GCN outputs
                # (DVE SBUF->SBUF bf16 runs in the fast 2x/4x mode)
                for d in range(2):
                    for c in range(3):
                        nc.vector.tensor_scalar(
                            pooled[d][c][:],
                            pooled_raw[d][c][:],
                            bc3[:, c : c + 1],
                            0.0,
                            mybir.AluOpType.add,
                            MAXOP,
                        )
                for d in range(2):
                    gfc = pool.tile([78, 2 * GPC], BF16, tag=f"gfc{d}", name=f"gfc{d}")
                    for m in range(2):
                        for n in range(2):
                            ps = psum.tile([78, 512], F32, tag="ps", name="ps")
                            for k in range(3):
                                nc.tensor.matmul(
                                    ps[:],
                                    wg1[:, k, m * 78 : (m + 1) * 78],
                                    pooled[d][k][:, n * 512 : (n + 1) * 512],
                                    start=(k == 0),
                                    stop=(k == 2),
                                )
                            nc.scalar.activation(
                                gfc[:, m * GPC + n * 512 : m * GPC + (n + 1) * 512],
                                ps[:],
                                RELU,
                                bias=bg1[:, m : m + 1],
                            )
                    for n in range(2):
                        ps = psum.tile([128, 512], F32, tag="ps", name="ps")
                        for k in range(2):
                            nc.tensor.matmul(
                                ps[:],
                                wg2[:, k, :],
                                gfc[:, k * GPC + n * 512 : k * GPC + (n + 1) * 512],
                                start=(k == 0),
                                stop=(k == 1),
                            )
                        nc.scalar.activation(
                            demb[d][:, n * 512 : (n + 1) * 512],
                            ps[:],
                            IDENT,
                            bias=bg2[:],
                        )

                # ---------------- head ----------------
                xcs = [demb[0], demb[1], c3T]
                hf1 = pool.tile([128, 2 * GPC], BF16, tag="hf1", name="hf1")
                for m in range(2):
                    for n in range(2):
                        ps = psum.tile([128, 512], F32, tag="ps", name="ps")
                        for k in range(3):
                            nc.tensor.matmul(
                                ps[:],
                                wf1[:, k, m * 128 : (m + 1) * 128],
                                xcs[k][:, n * 512 : (n + 1) * 512],
                                start=(k == 0),
                                stop=(k == 2),
                            )
                        nc.scalar.activation(
                            hf1[:, m * GPC + n * 512 : m * GPC + (n + 1) * 512],
                            ps[:],
                            RELU,
                            bias=bf1[:, m : m + 1],
                        )
                hf2 = pool.tile([128, GPC], BF16, tag="hf2", name="hf2")
                for n in range(2):
                    ps = psum.tile([128, 512], F32, tag="ps", name="ps")
                    for k in range(2):
                        nc.tensor.matmul(
                            ps[:],
                            wf2[:, k, :],
                            hf1[:, k * GPC + n * 512 : k * GPC + (n + 1) * 512],
                            start=(k == 0),
                            stop=(k == 1),
                        )
                    nc.scalar.activation(
                        hf2[:, n * 512 : (n + 1) * 512], ps[:], RELU, bias=bf2[:]
                    )
                osb = pool.tile([2, GPC], F32, tag="osb", name="osb")
                for n in range(2):
                    ps = psum.tile([2, 512], F32, tag="ps", name="ps")
                    nc.tensor.matmul(
                        ps[:],
                        wo[:],
                        hf2[:, n * 512 : (n + 1) * 512],
                        start=True,
                        stop=True,
                    )
                    nc.scalar.activation(
                        osb[:, n * 512 : (n + 1) * 512], ps[:], IDENT, bias=bo[:]
                    )
                nc.sync.dma_start(out_d[:], osb[:])

    nc.compile()
    return nc


def kernel(x1, edge_index1, batch1, x2, edge_index2, batch2, cell,
           Wc1, bc1, Wc2, bc2, Wc3, bc3, Wg1, bg1, Wg2, bg2,
           Wr1, br1, Wr2, br2, Wr3, br3, Wf1, bf1, Wf2, bf2, Wo, bo):
    if "nc" not in _CACHE:
        _CACHE["nc"] = _build_program()
    nc = _CACHE["nc"]

    g1p = _pack_group(x1, Wc1, edge_index1)
    g2p = _pack_group(x2, Wc1, edge_index2)
    cellc = _prep_cell(cell)

    bf = lambda a: np.asarray(a, dtype=np.float32).astype(NP_BF16)
    f32 = lambda a: np.asarray(a, dtype=np.float32)

    def padk(a, k=100):
        out = np.zeros((k,) + a.shape[1:], dtype=a.dtype)
        out[: a.shape[0]] = a
        return out

    # wc2 with the bias row at k=78
    wc2h = np.zeros((100, F2), dtype=NP_BF16)
    wc2h[0:F1] = bf(Wc2)
    wc2h[F1] = bf(bc2)

    # aug row for the ga tiles: per pair, cols 0:78 = b1, 78:178 = ones
    augrow = np.empty((1, GRP, 178), dtype=NP_BF16)
    augrow[..., 0:F1] = np.asarray(bc1, np.float32).astype(NP_BF16)
    augrow[..., F1:178] = NP_BF16(1.0)

    shared = dict(
        wc2=wc2h,
        wc3=padk(bf(_wchunk(np.asarray(Wc3, np.float32), 78))),
        augrow=augrow,
        wg1=bf(_wchunk(np.asarray(Wg1, np.float32), 104)),
        wg2=bf(_wchunk(np.asarray(Wg2, np.float32), 78)),
        wr1=bf(_wchunk(np.asarray(Wr1, np.float32), 106)),
        wr2=bf(_wchunk(np.asarray(Wr2, np.float32), 128)),
        wr3=bf(_wchunk(np.asarray(Wr3, np.float32), 128)),
        wf1=bf(_wchunk(np.asarray(Wf1, np.float32), 128)),
        wf2=bf(_wchunk(np.asarray(Wf2, np.float32), 128)),
        wo=bf(Wo),
        ones=np.ones((1, 8, 100), dtype=NP_BF16),
        bc3=_bchunk(f32(bc3), 3),
        bg1=_bchunk(f32(bg1), 2),
        bg2=f32(bg2).reshape(128, 1),
        br1=_bchunk(f32(br1), 4),
        br2=_bchunk(f32(br2), 2),
        br3=f32(br3).reshape(128, 1),
        bf1=_bchunk(f32(bf1), 2),
        bf2=f32(bf2).reshape(128, 1),
        bo=f32(bo).reshape(2, 1),
    )

    in_maps = []
    for c in range(NCORES):
        m = dict(shared)
        m["g1p"] = g1p[c]
        m["g2p"] = g2p[c]
        m["cellc"] = cellc[c]
        in_maps.append(m)

    res = run_bass_kernel_spmd(nc, in_maps, list(range(NCORES)))
    _CACHE["last_result"] = res
    out = np.concatenate(
        [np.asarray(res.results[c]["outT"], np.float32).T for c in range(NCORES)],
        axis=0,
    )
    return out
